# revision 5
# baseline (speedup 1.0000x reference)
"""Trainium2 Bass kernel: image -> 2-photon Fock-state basis change.

The reference op is `out[fock_idx] = input_state` with `out` zeros elsewhere
(fock_idx injective), i.e. a pure row scatter [36864, 512] -> [73920, 512].

fock_idx has block structure: input rows [i*192, (i+1)*192) land on output
rows [start(i), start(i)+192) contiguously, so the scatter is 192 contiguous
block copies plus zero fills — pure DMA work.

Sharding: data parallel along the batch dim. Each of the 8 cores gets a
contiguous 64-column slice and runs the identical SPMD program: DRAM->DRAM
block copies, pair-merged into 3D strided APs (two blocks per dma_start)
and split across the SP and ACT HWDGE rings to halve sequencer issue time.

Zero rows: the Bass runtime contract zero-initializes ExternalOutput
buffers (run_bass_kernel_spmd pre-zeros natively; the PJRT path feeds the
NEFF zero-filled output-named buffers), so unwritten rows are zero. kernel()
still validates this on the host and repairs + warns if it ever fails.
"""

import numpy as np

D1 = 192
D2 = 192
M = D1 + D2
IMG_DIM = D1 * D2            # 36864
FOCK_DIM = M * (M + 1) // 2  # 73920
BATCH = 512
N_CORES = 8
BS = BATCH // N_CORES        # 64 columns per core

# If True, emit zero-fill DMAs in the kernel instead of relying on
# pre-zeroed output buffers.
ZERO_FILL = False

ZW = 2316          # zero tile free dim (f32)
MAX_ZCHUNK = 2316  # rows per zero DMA: 2316*64 == 128*1158


def _fock_indices() -> np.ndarray:
    i = np.repeat(np.arange(D1), D2)
    j = np.tile(np.arange(D2), D1)
    q = D1 + j
    idx = i * M - i * (i - 1) // 2 + (q - i)
    return idx.astype(np.int32)


def _plan(fock_idx: np.ndarray):
    """Decompose the scatter into contiguous runs + zero intervals."""
    idx = np.asarray(fock_idx, dtype=np.int64).ravel()
    assert idx.shape[0] == IMG_DIM
    assert idx.min() >= 0 and idx.max() < FOCK_DIM
    assert np.unique(idx).size == IMG_DIM, "fock_idx must be injective"

    # maximal runs where consecutive input rows map to consecutive out rows
    brk = np.nonzero(np.diff(idx) != 1)[0] + 1
    starts_in = np.concatenate([[0], brk])
    ends_in = np.concatenate([brk, [IMG_DIM]])
    runs = [(int(a), int(idx[a]), int(b - a)) for a, b in zip(starts_in, ends_in)]
    assert len(runs) <= 1024, f"scatter too fragmented: {len(runs)} runs"

    # zero intervals = complement of scattered rows
    covered = np.zeros(FOCK_DIM, dtype=bool)
    covered[idx] = True
    d = np.diff(covered.astype(np.int8))
    zstarts = np.nonzero(d == -1)[0] + 1
    zends = np.nonzero(d == 1)[0] + 1
    if not covered[0]:
        zstarts = np.concatenate([[0], zstarts])
    if not covered[FOCK_DIM - 1]:
        zends = np.concatenate([zends, [FOCK_DIM]])
    zeros = [(int(a), int(b - a)) for a, b in zip(zstarts, zends)]
    n_covered = sum(r[2] for r in runs)
    n_zero = sum(z[1] for z in zeros)
    assert n_covered + n_zero == FOCK_DIM
    return runs, zeros


def _pair_runs(runs):
    """Pair equal-length runs: each pair becomes one 3D-AP dma_start."""
    from collections import defaultdict

    by_len = defaultdict(list)
    for r in runs:
        by_len[r[2]].append(r)
    pairs, singles = [], []
    for length, group in by_len.items():
        it = iter(group)
        for r in it:
            r2 = next(it, None)
            if r2 is None:
                singles.append(r)
            else:
                pairs.append((r, r2))
    return pairs, singles


def _build_program(runs, zeros):
    import concourse.bacc as bacc
    import concourse.bass as bass
    import concourse.tile as tile
    from concourse import mybir

    nc = bacc.Bacc("TRN2", debug=False, num_devices=N_CORES)
    x = nc.dram_tensor("x", [IMG_DIM, BS], mybir.dt.float32, kind="ExternalInput").ap()
    y = nc.dram_tensor(
        "y", [FOCK_DIM, BS], mybir.dt.float32, kind="ExternalOutput"
    ).ap()

    pairs, singles = _pair_runs(runs)
    engines = [lambda: nc.sync, lambda: nc.scalar]

    with tile.TileContext(nc) as tc:
        if ZERO_FILL:
            with tc.tile_pool(name="zeros", bufs=1) as zpool:
                ztile = zpool.tile([128, ZW], mybir.dt.float32)
                nc.vector.memset(ztile[:], 0.0)
                k = 0
                for r0, length in zeros:
                    r = r0
                    left = length
                    while left > 0:
                        c = min(left, MAX_ZCHUNK)
                        if c % 2 == 0:
                            src = ztile[0:128, 0 : (c * BS) // 128]
                        else:
                            src = ztile[0:64, 0 : (c * BS) // 64]
                        eng = engines[k % 2]()
                        k += 1
                        eng.dma_start(out=y[r : r + c, :], in_=src)
                        r += c
                        left -= c

        k = 0
        for (a1, b1, ln), (a2, b2, _) in pairs:
            el = ln * BS
            # last dim becomes the SDMA descriptor size and the descriptor
            # count drives the engine spread: aim for ~3KB descriptors.
            desc = next(
                (d for d in (768, 512, 256, 128, BS) if el % d == 0), BS
            )
            mid = el // desc
            in_ap = bass.AP(
                x.tensor,
                a1 * BS,
                [[(a2 - a1) * BS, 2], [desc, mid], [1, desc]],
            )
            out_ap = bass.AP(
                y.tensor,
                b1 * BS,
                [[(b2 - b1) * BS, 2], [desc, mid], [1, desc]],
            )
            eng = engines[k % 2]()
            k += 1
            eng.dma_start(out=out_ap, in_=in_ap)
        for a, b, ln in singles:
            eng = engines[k % 2]()
            k += 1
            eng.dma_start(out=y[b : b + ln, :], in_=x[a : a + ln, :])

    nc.compile()
    return nc


_cache = {}


def _get_program(fock_idx: np.ndarray):
    key = hash(np.asarray(fock_idx, dtype=np.int64).tobytes())
    if key not in _cache:
        runs, zeros = _plan(fock_idx)
        _cache[key] = (_build_program(runs, zeros), zeros)
    return _cache[key]


def _execute(x_full: np.ndarray, fock_idx: np.ndarray, trace=False, tmpdir=None):
    from concourse import bass_utils

    nc, zeros = _get_program(fock_idx)
    in_maps = [
        {"x": np.ascontiguousarray(x_full[:, c * BS : (c + 1) * BS])}
        for c in range(N_CORES)
    ]
    kw = {}
    if trace:
        kw = {"trace": True, "tmpdir": tmpdir}
    res = bass_utils.run_bass_kernel_spmd(nc, in_maps, list(range(N_CORES)), **kw)
    out = np.concatenate([res.results[c]["y"] for c in range(N_CORES)], axis=1)

    if not ZERO_FILL:
        # The runtime hands the NEFF zero-initialized output buffers, so
        # unwritten rows must be zero. Validate; repair on the host if the
        # contract is ever violated (should never happen).
        bad = 0
        for r0, length in zeros:
            seg = out[r0 : r0 + length]
            if seg.any():
                bad += int(np.count_nonzero(seg))
                seg[:] = 0
        if bad:
            import sys

            print(
                f"WARNING: output buffer was not zero-initialized "
                f"({bad} nonzero elems in zero rows); repaired on host",
                file=sys.stderr,
            )
    return out, res


def kernel(**inputs) -> np.ndarray:
    x_full = np.ascontiguousarray(np.asarray(inputs["input_state"], dtype=np.float32))
    assert x_full.shape == (IMG_DIM, BATCH)
    fock_idx = inputs.get("fock_idx")
    fock_idx = (
        _fock_indices() if fock_idx is None else np.asarray(fock_idx, dtype=np.int64)
    )
    out, _ = _execute(x_full, fock_idx)
    return out.astype(np.float32, copy=False)


# revision 7
# speedup vs baseline: 2.1820x; 2.1820x over previous
"""Trainium2 Bass kernel: image -> 2-photon Fock-state basis change.

The reference op is `out[fock_idx] = input_state` with `out` zeros elsewhere
(fock_idx injective), i.e. a pure row scatter [36864, 512] -> [73920, 512].

fock_idx has block structure: input rows [i*192, (i+1)*192) land on output
rows [start(i), start(i)+192) contiguously, so the scatter is 192 contiguous
block copies plus zero fills — pure DMA work.

Sharding: data parallel along the batch dim. Each of the 8 cores gets a
contiguous 64-column slice and runs the identical SPMD program: DRAM->DRAM
block copies, pair-merged into 3D strided APs (two blocks per dma_start)
and split across the SP and ACT HWDGE rings to halve sequencer issue time.

Zero rows: the Bass runtime contract zero-initializes ExternalOutput
buffers (run_bass_kernel_spmd pre-zeros natively; the PJRT path feeds the
NEFF zero-filled output-named buffers), so unwritten rows are zero. kernel()
still validates this on the host and repairs + warns if it ever fails.
"""

import numpy as np

D1 = 192
D2 = 192
M = D1 + D2
IMG_DIM = D1 * D2            # 36864
FOCK_DIM = M * (M + 1) // 2  # 73920
BATCH = 512
N_CORES = 8
BS = BATCH // N_CORES        # 64 columns per core

# If True, emit zero-fill DMAs in the kernel instead of relying on
# pre-zeroed output buffers.
ZERO_FILL = False

ZW = 2316          # zero tile free dim (f32)
MAX_ZCHUNK = 2316  # rows per zero DMA: 2316*64 == 128*1158


def _fock_indices() -> np.ndarray:
    i = np.repeat(np.arange(D1), D2)
    j = np.tile(np.arange(D2), D1)
    q = D1 + j
    idx = i * M - i * (i - 1) // 2 + (q - i)
    return idx.astype(np.int32)


def _plan(fock_idx: np.ndarray):
    """Decompose the scatter into contiguous runs + zero intervals."""
    idx = np.asarray(fock_idx, dtype=np.int64).ravel()
    assert idx.shape[0] == IMG_DIM
    assert idx.min() >= 0 and idx.max() < FOCK_DIM
    assert np.unique(idx).size == IMG_DIM, "fock_idx must be injective"

    # maximal runs where consecutive input rows map to consecutive out rows
    brk = np.nonzero(np.diff(idx) != 1)[0] + 1
    starts_in = np.concatenate([[0], brk])
    ends_in = np.concatenate([brk, [IMG_DIM]])
    runs = [(int(a), int(idx[a]), int(b - a)) for a, b in zip(starts_in, ends_in)]
    assert len(runs) <= 1024, f"scatter too fragmented: {len(runs)} runs"

    # zero intervals = complement of scattered rows
    covered = np.zeros(FOCK_DIM, dtype=bool)
    covered[idx] = True
    d = np.diff(covered.astype(np.int8))
    zstarts = np.nonzero(d == -1)[0] + 1
    zends = np.nonzero(d == 1)[0] + 1
    if not covered[0]:
        zstarts = np.concatenate([[0], zstarts])
    if not covered[FOCK_DIM - 1]:
        zends = np.concatenate([zends, [FOCK_DIM]])
    zeros = [(int(a), int(b - a)) for a, b in zip(zstarts, zends)]
    n_covered = sum(r[2] for r in runs)
    n_zero = sum(z[1] for z in zeros)
    assert n_covered + n_zero == FOCK_DIM
    return runs, zeros


def _pair_runs(runs):
    """Pair equal-length runs: each pair becomes one 3D-AP dma_start."""
    from collections import defaultdict

    by_len = defaultdict(list)
    for r in runs:
        by_len[r[2]].append(r)
    pairs, singles = [], []
    for length, group in by_len.items():
        it = iter(group)
        for r in it:
            r2 = next(it, None)
            if r2 is None:
                singles.append(r)
            else:
                pairs.append((r, r2))
    return pairs, singles


def _build_program(runs, zeros):
    import concourse.bacc as bacc
    import concourse.bass as bass
    import concourse.tile as tile
    from concourse import mybir

    nc = bacc.Bacc("TRN2", debug=False, num_devices=N_CORES)
    x = nc.dram_tensor("x", [IMG_DIM, BS], mybir.dt.float32, kind="ExternalInput").ap()
    y = nc.dram_tensor(
        "y", [FOCK_DIM, BS], mybir.dt.float32, kind="ExternalOutput"
    ).ap()

    engines = [lambda: nc.sync, lambda: nc.scalar]

    with tile.TileContext(nc) as tc:
        if ZERO_FILL:
            with tc.tile_pool(name="zeros", bufs=1) as zpool:
                ztile = zpool.tile([128, ZW], mybir.dt.float32)
                nc.vector.memset(ztile[:], 0.0)
                k = 0
                for r0, length in zeros:
                    r = r0
                    left = length
                    while left > 0:
                        c = min(left, MAX_ZCHUNK)
                        if c % 2 == 0:
                            src = ztile[0:128, 0 : (c * BS) // 128]
                        else:
                            src = ztile[0:64, 0 : (c * BS) // 64]
                        eng = engines[k % 2]()
                        k += 1
                        eng.dma_start(out=y[r : r + c, :], in_=src)
                        r += c
                        left -= c

        # One flat DRAM->DRAM dma_start per run: contiguous transfers split
        # evenly across all 16 SDMA engines (multi-dim APs do not), so keep
        # them flat and halve sequencer issue time by alternating between
        # the SP and ACT HWDGE rings.
        k = 0
        for a, b, ln in runs:
            eng = engines[k % 2]()
            k += 1
            eng.dma_start(out=y[b : b + ln, :], in_=x[a : a + ln, :])

    nc.compile()
    return nc


_cache = {}


def _get_program(fock_idx: np.ndarray):
    key = hash(np.asarray(fock_idx, dtype=np.int64).tobytes())
    if key not in _cache:
        runs, zeros = _plan(fock_idx)
        _cache[key] = (_build_program(runs, zeros), zeros)
    return _cache[key]


def _execute(x_full: np.ndarray, fock_idx: np.ndarray, trace=False, tmpdir=None):
    from concourse import bass_utils

    nc, zeros = _get_program(fock_idx)
    in_maps = [
        {"x": np.ascontiguousarray(x_full[:, c * BS : (c + 1) * BS])}
        for c in range(N_CORES)
    ]
    kw = {}
    if trace:
        kw = {"trace": True, "tmpdir": tmpdir}
    res = bass_utils.run_bass_kernel_spmd(nc, in_maps, list(range(N_CORES)), **kw)
    out = np.concatenate([res.results[c]["y"] for c in range(N_CORES)], axis=1)

    if not ZERO_FILL:
        # The runtime hands the NEFF zero-initialized output buffers, so
        # unwritten rows must be zero. Validate; repair on the host if the
        # contract is ever violated (should never happen).
        bad = 0
        for r0, length in zeros:
            seg = out[r0 : r0 + length]
            if seg.any():
                bad += int(np.count_nonzero(seg))
                seg[:] = 0
        if bad:
            import sys

            print(
                f"WARNING: output buffer was not zero-initialized "
                f"({bad} nonzero elems in zero rows); repaired on host",
                file=sys.stderr,
            )
    return out, res


def kernel(**inputs) -> np.ndarray:
    x_full = np.ascontiguousarray(np.asarray(inputs["input_state"], dtype=np.float32))
    assert x_full.shape == (IMG_DIM, BATCH)
    fock_idx = inputs.get("fock_idx")
    fock_idx = (
        _fock_indices() if fock_idx is None else np.asarray(fock_idx, dtype=np.int64)
    )
    out, _ = _execute(x_full, fock_idx)
    return out.astype(np.float32, copy=False)


# revision 9
# speedup vs baseline: 4.4755x; 2.0511x over previous
"""Trainium2 Bass kernel: image -> 2-photon Fock-state basis change.

The reference op is `out[fock_idx] = input_state` with `out` zeros elsewhere
(fock_idx injective), i.e. a pure row scatter [36864, 512] -> [73920, 512].

fock_idx has block structure: input rows [i*192, (i+1)*192) land on output
rows [start(i), start(i)+192) contiguously with start(i) quadratic in i, so
the scatter is 192 contiguous block copies plus zero fills — pure DMA work.

Sharding (fast path): split the *image rows* across the 8 cores — core k
copies blocks 24k..24k+23 with the full 512-wide batch, 384KB per flat
DRAM->DRAM dma_start, 24 instructions per core. The SPMD program stays
uniform by computing each core's output offsets from partition_id in
sequencer registers: local_row(j) = 192 + j*A - j(j-1)/2 with
A = 383 - 24*pid. Each core's output buffer is its slab of the Fock vector
(global rows [start(24k)-192, ...)); the host pastes slabs back together.

Zero rows are never written: the Bass runtime zero-initializes
ExternalOutput buffers (native path pre-zeros; the PJRT path feeds the NEFF
zero-filled buffers). kernel() validates this and repairs + warns if the
contract is ever violated.

A generic batch-sharded path (64 columns per core, one flat dma_start per
contiguous run, no partition_id math) handles any other injective fock_idx.
"""

import numpy as np

D1 = 192
D2 = 192
M = D1 + D2
IMG_DIM = D1 * D2            # 36864
FOCK_DIM = M * (M + 1) // 2  # 73920
BATCH = 512
N_CORES = 8
BS = BATCH // N_CORES        # batch-shard path: 64 columns per core

BPC = D1 // N_CORES          # row-shard path: 24 blocks per core
# uniform per-core output rows: 192 lead margin + largest slab
# (core 7: FOCK_DIM - start(168) = 23412 rows)
OUT_ROWS = 23604


def _fock_indices() -> np.ndarray:
    i = np.repeat(np.arange(D1), D2)
    j = np.tile(np.arange(D2), D1)
    q = D1 + j
    idx = i * M - i * (i - 1) // 2 + (q - i)
    return idx.astype(np.int32)


def _block_starts() -> np.ndarray:
    i = np.arange(D1, dtype=np.int64)
    return i * M - i * (i - 1) // 2 + (D1 - i)


# ---------------------------------------------------------------- planning


def _plan(fock_idx: np.ndarray):
    """Decompose the scatter into contiguous runs + zero intervals."""
    idx = np.asarray(fock_idx, dtype=np.int64).ravel()
    assert idx.shape[0] == IMG_DIM
    assert idx.min() >= 0 and idx.max() < FOCK_DIM
    assert np.unique(idx).size == IMG_DIM, "fock_idx must be injective"

    brk = np.nonzero(np.diff(idx) != 1)[0] + 1
    starts_in = np.concatenate([[0], brk])
    ends_in = np.concatenate([brk, [IMG_DIM]])
    runs = [(int(a), int(idx[a]), int(b - a)) for a, b in zip(starts_in, ends_in)]
    assert len(runs) <= 1024, f"scatter too fragmented: {len(runs)} runs"

    covered = np.zeros(FOCK_DIM, dtype=bool)
    covered[idx] = True
    d = np.diff(covered.astype(np.int8))
    zstarts = np.nonzero(d == -1)[0] + 1
    zends = np.nonzero(d == 1)[0] + 1
    if not covered[0]:
        zstarts = np.concatenate([[0], zstarts])
    if not covered[FOCK_DIM - 1]:
        zends = np.concatenate([zends, [FOCK_DIM]])
    zeros = [(int(a), int(b - a)) for a, b in zip(zstarts, zends)]
    assert sum(r[2] for r in runs) + sum(z[1] for z in zeros) == FOCK_DIM
    return runs, zeros


def _is_fock_pattern(runs) -> bool:
    if len(runs) != D1:
        return False
    starts = _block_starts()
    return all(
        a == i * D2 and ln == D2 and b == int(starts[i])
        for i, (a, b, ln) in enumerate(runs)
    )


# ---------------------------------------------------------------- programs


def _build_rowshard_program():
    import concourse.bacc as bacc
    import concourse.bass as bass
    import concourse.tile as tile
    from concourse import mybir

    nc = bacc.Bacc("TRN2", debug=False, num_devices=N_CORES)
    rows_in = BPC * D2  # 4608
    x = nc.dram_tensor(
        "x", [rows_in, BATCH], mybir.dt.float32, kind="ExternalInput"
    ).ap()
    y = nc.dram_tensor(
        "y", [OUT_ROWS, BATCH], mybir.dt.float32, kind="ExternalOutput"
    ).ap()
    starts = _block_starts()

    with tile.TileContext(nc) as tc:
        engs = [nc.sync, nc.scalar]
        # per-engine snapped A = 383 - 24*pid
        A = []
        for eng in engs:
            pid = eng.partition_id()
            A.append(eng.snap(383 - pid * BPC))
        for j in range(BPC):
            eng = engs[j % 2]
            Aj = A[j % 2]
            tj = j * (j - 1) // 2
            off_rows = Aj * j + (D2 - tj)
            dyn = y[bass.ds(off_rows, D2), :]
            # true core-0 offsets double as a valid disjointness claim for
            # Tile's dependency tracking (blocks never overlap on any core)
            out_ap = bass.AP(
                tensor=dyn.tensor,
                offset=dyn.offset,
                ap=dyn.ap,
                dep_tracking_offset=int(starts[j]) * BATCH,
            )
            eng.dma_start(out=out_ap, in_=x[j * D2 : (j + 1) * D2, :])
    nc.compile()
    return nc


def _build_batchshard_program(runs):
    import concourse.bacc as bacc
    import concourse.tile as tile
    from concourse import mybir

    nc = bacc.Bacc("TRN2", debug=False, num_devices=N_CORES)
    x = nc.dram_tensor("x", [IMG_DIM, BS], mybir.dt.float32, kind="ExternalInput").ap()
    y = nc.dram_tensor(
        "y", [FOCK_DIM, BS], mybir.dt.float32, kind="ExternalOutput"
    ).ap()

    with tile.TileContext(nc) as tc:
        engines = [nc.sync, nc.scalar]
        for k, (a, b, ln) in enumerate(runs):
            engines[k % 2].dma_start(out=y[b : b + ln, :], in_=x[a : a + ln, :])
    nc.compile()
    return nc


_cache = {}


def _get_program(fock_idx: np.ndarray):
    key = hash(np.asarray(fock_idx, dtype=np.int64).tobytes())
    if key not in _cache:
        runs, zeros = _plan(fock_idx)
        if _is_fock_pattern(runs):
            _cache[key] = ("row", _build_rowshard_program(), zeros)
        else:
            _cache[key] = ("batch", _build_batchshard_program(runs), zeros)
    return _cache[key]


# ---------------------------------------------------------------- execution


def _run(nc, in_maps, trace=False, tmpdir=None):
    from concourse import bass_utils

    kw = {"trace": True, "tmpdir": tmpdir} if trace else {}
    return bass_utils.run_bass_kernel_spmd(nc, in_maps, list(range(N_CORES)), **kw)


def _execute(x_full: np.ndarray, fock_idx: np.ndarray, trace=False, tmpdir=None):
    mode, nc, zeros = _get_program(fock_idx)

    if mode == "row":
        rows_in = BPC * D2
        in_maps = [
            {"x": x_full[c * rows_in : (c + 1) * rows_in]} for c in range(N_CORES)
        ]
        res = _run(nc, in_maps, trace, tmpdir)
        starts = _block_starts()
        out = np.zeros((FOCK_DIM, BATCH), dtype=np.float32)
        for k in range(N_CORES):
            g0 = int(starts[BPC * k])
            g1 = int(starts[BPC * (k + 1)]) if k < N_CORES - 1 else FOCK_DIM
            out[g0:g1] = res.results[k]["y"][D2 : D2 + (g1 - g0)]
    else:
        in_maps = [
            {"x": np.ascontiguousarray(x_full[:, c * BS : (c + 1) * BS])}
            for c in range(N_CORES)
        ]
        res = _run(nc, in_maps, trace, tmpdir)
        out = np.concatenate([res.results[c]["y"] for c in range(N_CORES)], axis=1)

    # The runtime hands the NEFF zero-initialized output buffers, so
    # unwritten rows must be zero. Validate; repair on the host if the
    # contract is ever violated (should never happen).
    bad = 0
    for r0, length in zeros:
        seg = out[r0 : r0 + length]
        if seg.any():
            bad += int(np.count_nonzero(seg))
            seg[:] = 0
    if bad:
        import sys

        print(
            f"WARNING: output buffer was not zero-initialized "
            f"({bad} nonzero elems in zero rows); repaired on host",
            file=sys.stderr,
        )
    return out, res


def kernel(**inputs) -> np.ndarray:
    x_full = np.ascontiguousarray(np.asarray(inputs["input_state"], dtype=np.float32))
    assert x_full.shape == (IMG_DIM, BATCH)
    fock_idx = inputs.get("fock_idx")
    fock_idx = (
        _fock_indices() if fock_idx is None else np.asarray(fock_idx, dtype=np.int64)
    )
    out, _ = _execute(x_full, fock_idx)
    return out.astype(np.float32, copy=False)


# revision 10
# speedup vs baseline: 4.4873x; 1.0026x over previous
"""Trainium2 Bass kernel: image -> 2-photon Fock-state basis change.

The reference op is `out[fock_idx] = input_state` with `out` zeros elsewhere
(fock_idx injective), i.e. a pure row scatter [36864, 512] -> [73920, 512].

fock_idx has block structure: input rows [i*192, (i+1)*192) land on output
rows [start(i), start(i)+192) contiguously with start(i) quadratic in i, so
the scatter is 192 contiguous block copies plus zero fills — pure DMA work.

Sharding (fast path): split the *image rows* across the 8 cores — core k
copies blocks 24k..24k+23 with the full 512-wide batch, 384KB per flat
DRAM->DRAM dma_start, 24 instructions per core. The SPMD program stays
uniform by computing each core's output offsets from partition_id in
sequencer registers: local_row(j) = 192 + j*A - j(j-1)/2 with
A = 383 - 24*pid. Each core's output buffer is its slab of the Fock vector
(global rows [start(24k)-192, ...)); the host pastes slabs back together.

Zero rows are never written: the Bass runtime zero-initializes
ExternalOutput buffers (native path pre-zeros; the PJRT path feeds the NEFF
zero-filled buffers). kernel() validates this and repairs + warns if the
contract is ever violated.

A generic batch-sharded path (64 columns per core, one flat dma_start per
contiguous run, no partition_id math) handles any other injective fock_idx.
"""

import numpy as np

D1 = 192
D2 = 192
M = D1 + D2
IMG_DIM = D1 * D2            # 36864
FOCK_DIM = M * (M + 1) // 2  # 73920
BATCH = 512
N_CORES = 8
BS = BATCH // N_CORES        # batch-shard path: 64 columns per core

BPC = D1 // N_CORES          # row-shard path: 24 blocks per core
# uniform per-core output rows: 192 lead margin + largest slab
# (core 7: FOCK_DIM - start(168) = 23412 rows)
OUT_ROWS = 23604


def _fock_indices() -> np.ndarray:
    i = np.repeat(np.arange(D1), D2)
    j = np.tile(np.arange(D2), D1)
    q = D1 + j
    idx = i * M - i * (i - 1) // 2 + (q - i)
    return idx.astype(np.int32)


def _block_starts() -> np.ndarray:
    i = np.arange(D1, dtype=np.int64)
    return i * M - i * (i - 1) // 2 + (D1 - i)


# ---------------------------------------------------------------- planning


def _plan(fock_idx: np.ndarray):
    """Decompose the scatter into contiguous runs + zero intervals."""
    idx = np.asarray(fock_idx, dtype=np.int64).ravel()
    assert idx.shape[0] == IMG_DIM
    assert idx.min() >= 0 and idx.max() < FOCK_DIM
    assert np.unique(idx).size == IMG_DIM, "fock_idx must be injective"

    brk = np.nonzero(np.diff(idx) != 1)[0] + 1
    starts_in = np.concatenate([[0], brk])
    ends_in = np.concatenate([brk, [IMG_DIM]])
    runs = [(int(a), int(idx[a]), int(b - a)) for a, b in zip(starts_in, ends_in)]
    assert len(runs) <= 1024, f"scatter too fragmented: {len(runs)} runs"

    covered = np.zeros(FOCK_DIM, dtype=bool)
    covered[idx] = True
    d = np.diff(covered.astype(np.int8))
    zstarts = np.nonzero(d == -1)[0] + 1
    zends = np.nonzero(d == 1)[0] + 1
    if not covered[0]:
        zstarts = np.concatenate([[0], zstarts])
    if not covered[FOCK_DIM - 1]:
        zends = np.concatenate([zends, [FOCK_DIM]])
    zeros = [(int(a), int(b - a)) for a, b in zip(zstarts, zends)]
    assert sum(r[2] for r in runs) + sum(z[1] for z in zeros) == FOCK_DIM
    return runs, zeros


def _is_fock_pattern(runs) -> bool:
    if len(runs) != D1:
        return False
    starts = _block_starts()
    return all(
        a == i * D2 and ln == D2 and b == int(starts[i])
        for i, (a, b, ln) in enumerate(runs)
    )


# ---------------------------------------------------------------- programs


def _build_rowshard_program():
    """Raw bacc kernel (no Tile): 12 dynamic-offset DMAs per HWDGE engine,
    one semaphore wait per engine at the end. Avoids Tile's preamble/tail
    barriers and its 8-deep DMA in-flight cap — the HWDGE rings provide
    hardware backpressure."""
    import concourse.bacc as bacc
    import concourse.bass as bass
    from concourse import mybir

    nc = bacc.Bacc("TRN2", debug=False, num_devices=N_CORES)
    rows_in = BPC * D2  # 4608
    x = nc.dram_tensor(
        "x", [rows_in, BATCH], mybir.dt.float32, kind="ExternalInput"
    ).ap()
    y = nc.dram_tensor(
        "y", [OUT_ROWS, BATCH], mybir.dt.float32, kind="ExternalOutput"
    ).ap()

    with (
        nc.semaphore("dma_sp") as s_sp,
        nc.semaphore("dma_act") as s_act,
        nc.Block() as block,
    ):

        def body(eng, sem, jstart):
            pid = eng.partition_id()
            A = eng.snap(383 - pid * BPC)
            n = 0
            for j in range(jstart, BPC, 2):
                tj = j * (j - 1) // 2
                off_rows = A * j + (D2 - tj)
                eng.dma_start(
                    out=y[bass.ds(off_rows, D2), :],
                    in_=x[j * D2 : (j + 1) * D2, :],
                ).then_inc(sem, 16)
                n += 1
            eng.wait_ge(sem, 16 * n)

        @block.sync
        def _(sync):
            body(sync, s_sp, 0)

        @block.scalar
        def _(scalar):
            body(scalar, s_act, 1)

    nc.compile()
    return nc


def _build_batchshard_program(runs):
    import concourse.bacc as bacc
    import concourse.tile as tile
    from concourse import mybir

    nc = bacc.Bacc("TRN2", debug=False, num_devices=N_CORES)
    x = nc.dram_tensor("x", [IMG_DIM, BS], mybir.dt.float32, kind="ExternalInput").ap()
    y = nc.dram_tensor(
        "y", [FOCK_DIM, BS], mybir.dt.float32, kind="ExternalOutput"
    ).ap()

    with tile.TileContext(nc) as tc:
        engines = [nc.sync, nc.scalar]
        for k, (a, b, ln) in enumerate(runs):
            engines[k % 2].dma_start(out=y[b : b + ln, :], in_=x[a : a + ln, :])
    nc.compile()
    return nc


_cache = {}


def _get_program(fock_idx: np.ndarray):
    key = hash(np.asarray(fock_idx, dtype=np.int64).tobytes())
    if key not in _cache:
        runs, zeros = _plan(fock_idx)
        if _is_fock_pattern(runs):
            _cache[key] = ("row", _build_rowshard_program(), zeros)
        else:
            _cache[key] = ("batch", _build_batchshard_program(runs), zeros)
    return _cache[key]


# ---------------------------------------------------------------- execution


def _run(nc, in_maps, trace=False, tmpdir=None):
    from concourse import bass_utils

    kw = {"trace": True, "tmpdir": tmpdir} if trace else {}
    return bass_utils.run_bass_kernel_spmd(nc, in_maps, list(range(N_CORES)), **kw)


def _execute(x_full: np.ndarray, fock_idx: np.ndarray, trace=False, tmpdir=None):
    mode, nc, zeros = _get_program(fock_idx)

    if mode == "row":
        rows_in = BPC * D2
        in_maps = [
            {"x": x_full[c * rows_in : (c + 1) * rows_in]} for c in range(N_CORES)
        ]
        res = _run(nc, in_maps, trace, tmpdir)
        starts = _block_starts()
        out = np.zeros((FOCK_DIM, BATCH), dtype=np.float32)
        for k in range(N_CORES):
            g0 = int(starts[BPC * k])
            g1 = int(starts[BPC * (k + 1)]) if k < N_CORES - 1 else FOCK_DIM
            out[g0:g1] = res.results[k]["y"][D2 : D2 + (g1 - g0)]
    else:
        in_maps = [
            {"x": np.ascontiguousarray(x_full[:, c * BS : (c + 1) * BS])}
            for c in range(N_CORES)
        ]
        res = _run(nc, in_maps, trace, tmpdir)
        out = np.concatenate([res.results[c]["y"] for c in range(N_CORES)], axis=1)

    # The runtime hands the NEFF zero-initialized output buffers, so
    # unwritten rows must be zero. Validate; repair on the host if the
    # contract is ever violated (should never happen).
    bad = 0
    for r0, length in zeros:
        seg = out[r0 : r0 + length]
        if seg.any():
            bad += int(np.count_nonzero(seg))
            seg[:] = 0
    if bad:
        import sys

        print(
            f"WARNING: output buffer was not zero-initialized "
            f"({bad} nonzero elems in zero rows); repaired on host",
            file=sys.stderr,
        )
    return out, res


def kernel(**inputs) -> np.ndarray:
    x_full = np.ascontiguousarray(np.asarray(inputs["input_state"], dtype=np.float32))
    assert x_full.shape == (IMG_DIM, BATCH)
    fock_idx = inputs.get("fock_idx")
    fock_idx = (
        _fock_indices() if fock_idx is None else np.asarray(fock_idx, dtype=np.int64)
    )
    out, _ = _execute(x_full, fock_idx)
    return out.astype(np.float32, copy=False)
